# revision 6
# baseline (speedup 1.0000x reference)
"""Trainium2 Bass kernel for nn_Consistent_loss_up_2 (scatter_memory).

Reference computation:
    bins = round(up*50+110) clipped to [0,255]; mask = up >= 0.0235
    scatter-max over i into up2left/up2right[k, 0, j, bin]:
        i > 128:  value (i-128)/60  -> up2right
        i <= 128: value (128-i)/60  -> up2left
    loss = mean(|up2right-right| masked) + mean(|up2left-left| masked)
    where masked = (d < 0.2) & (map != 0)

Key structure exploited (v2):
  * only bins 111..160 (50 of 256) are reachable -> per-k table block of
    width 102: [trashL][50 L bins][trashR][50 R bins]
  * scatter values are monotone in i, so scatter-max == overwrite-scatter
    in the right stream order (left: i descending via anti-diagonal
    "identity"; right: i ascending); gpsimd local_scatter is
    last-write-wins (verified on HW)
  * the f32->i16 convert on ACT rounds to nearest-even (verified on HW),
    which matches jnp.round exactly -> the bin rounding is free
  * per-k table offsets (102*k) + right-half offset (+51) are injected
    via a rank-1 PSUM prefill matmul; transposes accumulate on top,
    so ACT/DVE index math is batched and k-independent
  * masked points multiply to 0 and land on the per-k trash column;
    i=128 is killed by zeroing identR[0,0]
  * tables store 60*map + 256 in i16; empty bins (0) auto-fail the
    d<0.2 test; loss fused as |tbl - (60 ref)| threshold + accum_out

Sharding: data-parallel over batch B=128 across 8 cores (16 each).

Engine budget per core: gpsimd scatter ~42us (bottleneck, throughput
bound at ~9.6 cyc per 2x16 idx SIMD iter), ACT ~21us, DVE ~14us,
PE ~20us, DMA ~16us.
"""

import numpy as np

from concourse import bacc, mybir, tile
from concourse.bass_utils import run_bass_kernel_spmd

B, H, W = 128, 256, 256
NCORES = 8
KPC = B // NCORES   # batches per core = 16
GK = 4              # batches per psum group
NG = KPC // GK      # 4 groups
TBLK = 102          # per-k table block
GRPW = GK * TBLK    # 408 (table cols per scatter call)
REFW = KPC * TBLK   # 1632
PSW = GK * 2 * 128  # 1024 psum cols per (jt, group)
OFF = 256.0         # i16 table value offset (empty-bin sentinel)
THR = 0.0235

_cache = {}


def _build_bass():
    nc = bacc.Bacc("TRN2", target_bir_lowering=False)
    f32, f16, i16 = mybir.dt.float32, mybir.dt.float16, mybir.dt.int16
    Alu = mybir.AluOpType
    Act = mybir.ActivationFunctionType

    up_in = nc.dram_tensor("up_in", [KPC * H, W], f32, kind="ExternalInput")
    refs_in = nc.dram_tensor("refs_in", [W, REFW], f16, kind="ExternalInput")
    vee_in = nc.dram_tensor("vee_in", [128, PSW], i16, kind="ExternalInput")
    idl_in = nc.dram_tensor("idl_in", [128, 128], f16, kind="ExternalInput")
    idr_in = nc.dram_tensor("idr_in", [128, 128], f16, kind="ExternalInput")
    ones_in = nc.dram_tensor("ones_in", [1, 128], f16, kind="ExternalInput")
    patt_in = nc.dram_tensor("patt_in", [1, PSW], f16, kind="ExternalInput")
    out = nc.dram_tensor("out", [128, 2], f32, kind="ExternalOutput")

    with tile.TileContext(nc) as tc:
        with (
            tc.tile_pool(name="const", bufs=1) as constp,
            tc.tile_pool(name="stage", bufs=1) as stagep,
            tc.tile_pool(name="work", bufs=3) as workp,
            tc.tile_pool(name="stp", bufs=2) as stp,
            tc.tile_pool(name="psum", bufs=1, space="PSUM") as psump,
            tc.tile_pool(name="loss", bufs=1) as lossp,
        ):
            vee = constp.tile([128, PSW], i16)
            nc.sync.dma_start(vee[:], vee_in[:])
            idl = constp.tile([128, 128], f16)
            nc.sync.dma_start(idl[:], idl_in[:])
            idr = constp.tile([128, 128], f16)
            nc.sync.dma_start(idr[:], idr_in[:])
            ones1 = constp.tile([1, 128], f16)
            nc.sync.dma_start(ones1[:], ones_in[:])
            patt = constp.tile([1, PSW], f16)
            nc.sync.dma_start(patt[:], patt_in[:])
            noff = constp.tile([128, 1], f32)
            nc.vector.memset(noff[:], -OFF)

            refs_sb = []
            tbl = []
            for jt in range(2):
                r = stagep.tile([128, REFW], f16, tag=f"refs{jt}")
                nc.scalar.dma_start(r[:], refs_in[jt * 128:(jt + 1) * 128, :])
                refs_sb.append(r)
                tbl.append(
                    stagep.tile([128, REFW], i16, tag=f"tbl{jt}", name=f"tbl{jt}")
                )

            for g in range(NG):
                ps = []
                for jt in range(2):
                    p = psump.tile(
                        [128, PSW], f32, tag=f"ps{jt}{g % 2}", space="PSUM"
                    )
                    ps.append(p)
                    # rank-1 prefill: per-k table offset 102*kl, +51 for the
                    # right half; masked points (0) then land on trash cols
                    for c in range(0, PSW, 512):
                        nc.tensor.matmul(
                            p[:, c:c + 512], ones1[:], patt[:, c:c + 512],
                            start=True, stop=False, skip_group_check=True,
                        )

                for kl in range(GK):
                    k = g * GK + kl
                    ut = workp.tile([128, 2, W], f32, tag="ut")
                    src = up_in[k * H:(k + 1) * H, :].rearrange(
                        "(h p) w -> p h w", h=2
                    )
                    nc.sync.dma_start(ut[:], src)
                    utm = ut[:].rearrange("p h w -> p (h w)")

                    # fm = 50*u (fp16); bins come from the RNE i16 convert
                    fm = workp.tile([128, 2 * W], f16, tag="fm")
                    nc.scalar.activation(fm[:], utm, Act.Copy, scale=50.0)
                    # ixm = (u >= thr) * fm : masked -> 0 -> trash col
                    ixm = workp.tile([128, 2 * W], f16, tag="ixm")
                    nc.vector.scalar_tensor_tensor(
                        ixm[:], utm, THR, fm[:], op0=Alu.is_ge, op1=Alu.mult
                    )

                    for jt in range(2):
                        base = kl * 256
                        # left: h=0 rows (i=p), stream reversed (i desc)
                        nc.tensor.matmul(
                            ps[jt][:, base:base + 128],
                            ixm[:, jt * 128:jt * 128 + 128],
                            idl[:],
                            start=False, stop=False, skip_group_check=True,
                        )
                        # right: h=1 rows (i=128+p), idr[0,0]=0 kills i=128
                        nc.tensor.matmul(
                            ps[jt][:, base + 128:base + 256],
                            ixm[:, 256 + jt * 128:256 + jt * 128 + 128],
                            idr[:],
                            start=False, stop=(kl == GK - 1),
                            skip_group_check=True,
                        )

                for jt in range(2):
                    st = stp.tile([128, PSW], i16, tag=f"st{jt}")
                    nc.scalar.activation(st[:], ps[jt][:], Act.Copy)
                    nc.gpsimd.local_scatter(
                        tbl[jt][:, g * GRPW:(g + 1) * GRPW],
                        vee[:],
                        st[:],
                        channels=128,
                        num_elems=GRPW,
                        num_idxs=PSW,
                    )

            for jt in range(2):
                # t/a stay f32: quantizing t to fp16 puts a on a 0.25 grid
                # that contains the threshold 12.0 exactly, and the strict
                # a<12 test then drops the whole (11.875, 12) band (-1.7%)
                t = lossp.tile([128, REFW], f32, tag=f"t{jt}")
                nc.vector.tensor_tensor(
                    out=t[:], in0=tbl[jt][:], in1=refs_sb[jt][:],
                    op=Alu.subtract,
                )
                a = lossp.tile([128, REFW], f32, tag=f"a{jt}")
                nc.scalar.activation(a[:], t[:], Act.Abs, bias=noff[:, :])
                m = lossp.tile([128, REFW], f16, tag=f"m{jt}")
                part = lossp.tile([128, 1], f32, tag=f"part{jt}")
                nc.vector.scalar_tensor_tensor(
                    m[:], a[:], 12.0, a[:], op0=Alu.is_lt, op1=Alu.mult,
                    accum_out=part[:],
                )
                nc.scalar.dma_start(out[:, jt:jt + 1], part[:])

    nc.compile()
    return nc


def _host_constants():
    n = np.arange(128)
    veeL = (n + 1 + int(OFF)).astype(np.int16)
    veeR = (n + int(OFF)).astype(np.int16)
    vee256 = np.concatenate([veeL, veeR])
    vee = np.ascontiguousarray(
        np.broadcast_to(np.tile(vee256, GK), (128, PSW))
    ).astype(np.int16)

    ident = np.eye(128, dtype=np.float16)
    idl = np.ascontiguousarray(ident[::-1, :])  # anti-diagonal
    idr = ident.copy()
    idr[0, 0] = 0.0  # kill i=128

    ones1 = np.ones((1, 128), np.float16)
    npos = np.arange(PSW)
    patt = (TBLK * (npos // 256) + 51 * ((npos % 256) >= 128)).astype(
        np.float16
    )[None, :]
    return vee, idl, idr, ones1, np.ascontiguousarray(patt)


def _prep_refs(left, right):
    """[W, REFW] fp16 per core: row (jt*128+j), col 102*k + c:
    c=0,51 trash(4000); c in [1,50] left bins 111..160 (x60);
    c in [52,101] right bins 111..160 (x60)."""
    lft = left[:, 0, :, 111:161]   # [B, W, 50]
    rgt = right[:, 0, :, 111:161]
    refs = np.full((NCORES, KPC, W, TBLK), 4000.0, np.float32)
    refs[..., 1:51] = 60.0 * lft.reshape(NCORES, KPC, W, 50)
    refs[..., 52:102] = 60.0 * rgt.reshape(NCORES, KPC, W, 50)
    # -> [core, jt*128+j, k*102+c]
    refs = refs.transpose(0, 2, 1, 3).reshape(NCORES, W, REFW)
    return np.ascontiguousarray(refs.astype(np.float16))


def make_in_maps(up, left, right):
    up = np.asarray(up, np.float32)
    left = np.asarray(left, np.float32)
    right = np.asarray(right, np.float32)
    vee, idl, idr, ones1, patt = _host_constants()
    refs = _prep_refs(left, right)
    in_maps = []
    for c in range(NCORES):
        upc = np.ascontiguousarray(
            up[c * KPC:(c + 1) * KPC, 0].reshape(KPC * H, W)
        )
        in_maps.append({
            "up_in": upc,
            "refs_in": refs[c],
            "vee_in": vee,
            "idl_in": idl,
            "idr_in": idr,
            "ones_in": ones1,
            "patt_in": patt,
        })
    return in_maps


def get_nc():
    if "nc" not in _cache:
        _cache["nc"] = _build_bass()
    return _cache["nc"]


def reduce_results(results):
    total = 0.0
    for r in results:
        total += float(r["out"].astype(np.float64).sum())
    return np.float32(total / (60.0 * B * W * W))


def kernel(up, left, right):
    nc = get_nc()
    in_maps = make_in_maps(up, left, right)
    res = run_bass_kernel_spmd(nc, in_maps, core_ids=list(range(NCORES)))
    return reduce_results(res.results)


# revision 12
# speedup vs baseline: 1.0286x; 1.0286x over previous
"""Trainium2 Bass kernel for nn_Consistent_loss_up_2 (scatter_memory).

Reference computation:
    bins = round(up*50+110) clipped to [0,255]; mask = up >= 0.0235
    scatter-max over i into up2left/up2right[k, 0, j, bin]:
        i > 128:  value (i-128)/60  -> up2right
        i <= 128: value (128-i)/60  -> up2left
    loss = mean(|up2right-right| masked) + mean(|up2left-left| masked)
    where masked = (d < 0.2) & (map != 0)

Key structure exploited (v3):
  * only bins 111..160 (50 of 256) are reachable -> per-k table block of
    width 102: [trashL][50 L bins][trashR][50 R bins]
  * scatter values are monotone in i, so scatter-max == overwrite-scatter
    in the right stream order (left: i descending via anti-diagonal
    "identity"; right: i ascending); gpsimd local_scatter is
    last-write-wins (verified on HW)
  * the f32->i16 convert on ACT rounds to nearest-even (verified on HW),
    which matches jnp.round exactly -> the bin rounding is free
  * per-k table offsets (102*k) + right-half offset (+51) are injected
    via a rank-1 PSUM prefill matmul; transposes accumulate on top,
    so the index math is batched and k-independent
  * masked points multiply to 0 and land on the per-k trash column;
    i=128 is killed by zeroing identR[0,0]
  * tables store 60*map + 256 in i16; refs stay f32 (any coarser
    quantization of refs puts the d<0.2 threshold on the quantization
    grid, and the strict < then drops a one-sided band of the largest
    contributors: -1.7%..-2% observed with fp16 and x4-int refs)
  * loss: tbl -> f32 on ACT, t = tbl - refs (TT), a = |t - 256| (ACT),
    then sum(a*[a<12]) = sum(min(a,12)) - 12*(TOT-N) via two
    tensor_scalar+accum ops (out=(in op0 s1); accum=reduce(out,op1,s2));
    no slow scalar_tensor_tensor anywhere
  * emission is software-pipelined: group g's convert+scatter are
    emitted during group g+1's per-k work, so the ACT queue never
    head-of-line blocks the next group's fm ops

Sharding: data-parallel over batch B=128 across 8 cores (16 each).
"""

import numpy as np

from concourse import bacc, mybir, tile
from concourse.bass_utils import run_bass_kernel_spmd

B, H, W = 128, 256, 256
NCORES = 8
KPC = B // NCORES   # batches per core = 16
GK = 4              # batches per psum group
NG = KPC // GK      # 4 groups
TBLK = 102          # per-k table block
GRPW = GK * TBLK    # 408 (table cols per scatter call)
REFW = KPC * TBLK   # 1632
PSW = GK * 2 * 128  # 1024 psum cols per (jt, group)
OFF = 256.0         # i16 table value offset (empty-bin sentinel)
THR = 0.0235

_cache = {}


def _build_bass():
    nc = bacc.Bacc("TRN2", target_bir_lowering=False)
    f32, f16, i16 = mybir.dt.float32, mybir.dt.float16, mybir.dt.int16
    Alu = mybir.AluOpType
    Act = mybir.ActivationFunctionType

    up_in = nc.dram_tensor("up_in", [KPC * H, W], f32, kind="ExternalInput")
    refs_in = nc.dram_tensor("refs_in", [W, REFW], f32, kind="ExternalInput")
    vee_in = nc.dram_tensor("vee_in", [128, PSW], i16, kind="ExternalInput")
    idl_in = nc.dram_tensor("idl_in", [128, 128], f16, kind="ExternalInput")
    idr_in = nc.dram_tensor("idr_in", [128, 128], f16, kind="ExternalInput")
    ones_in = nc.dram_tensor("ones_in", [1, 128], f16, kind="ExternalInput")
    patt_in = nc.dram_tensor("patt_in", [1, PSW], f16, kind="ExternalInput")
    out = nc.dram_tensor("out", [128, 4], f32, kind="ExternalOutput")

    with tile.TileContext(nc) as tc:
        with (
            tc.tile_pool(name="const", bufs=1) as constp,
            tc.tile_pool(name="stage", bufs=1) as stagep,
            tc.tile_pool(name="work", bufs=3) as workp,
            tc.tile_pool(name="stp", bufs=2) as stp,
            tc.tile_pool(name="psum", bufs=1, space="PSUM") as psump,
            tc.tile_pool(name="loss", bufs=1) as lossp,
        ):
            vee = constp.tile([128, PSW], i16)
            nc.sync.dma_start(vee[:], vee_in[:])
            idl = constp.tile([128, 128], f16)
            nc.sync.dma_start(idl[:], idl_in[:])
            idr = constp.tile([128, 128], f16)
            nc.sync.dma_start(idr[:], idr_in[:])
            ones1 = constp.tile([1, 128], f16)
            nc.sync.dma_start(ones1[:], ones_in[:])
            patt = constp.tile([1, PSW], f16)
            nc.sync.dma_start(patt[:], patt_in[:])
            noff = constp.tile([128, 1], f32)
            nc.vector.memset(noff[:], -OFF)

            refs_sb = []
            tbl = []
            for jt in range(2):
                r = stagep.tile([128, REFW], f32, tag=f"refs{jt}")
                nc.scalar.dma_start(r[:], refs_in[jt * 128:(jt + 1) * 128, :])
                refs_sb.append(r)
                tbl.append(
                    stagep.tile([128, REFW], i16, tag=f"tbl{jt}", name=f"tbl{jt}")
                )

            def emit_group_tail(g, ps):
                for jt in range(2):
                    st = stp.tile([128, PSW], i16, tag=f"st{jt}")
                    nc.scalar.activation(st[:], ps[jt][:], Act.Copy)
                    nc.gpsimd.local_scatter(
                        tbl[jt][:, g * GRPW:(g + 1) * GRPW],
                        vee[:],
                        st[:],
                        channels=128,
                        num_elems=GRPW,
                        num_idxs=PSW,
                    )

            pend = None
            for g in range(NG):
                ps = []
                for jt in range(2):
                    p = psump.tile(
                        [128, PSW], f32, tag=f"ps{jt}{g % 2}", space="PSUM"
                    )
                    ps.append(p)
                    # rank-1 prefill: per-k table offset 102*kl, +51 for
                    # the right half (and the trash routing for zeros)
                    for c in range(0, PSW, 512):
                        nc.tensor.matmul(
                            p[:, c:c + 512], ones1[:], patt[:, c:c + 512],
                            start=True, stop=False, skip_group_check=True,
                        )

                for kl in range(GK):
                    k = g * GK + kl
                    ut = workp.tile([128, 2, W], f32, tag="ut")
                    src = up_in[k * H:(k + 1) * H, :].rearrange(
                        "(h p) w -> p h w", h=2
                    )
                    nc.sync.dma_start(ut[:], src)
                    utm = ut[:].rearrange("p h w -> p (h w)")

                    # fm = 50*u (fp16); bins come from the RNE i16 convert
                    fm = workp.tile([128, 2 * W], f16, tag="fm")
                    nc.scalar.activation(fm[:], utm, Act.Copy, scale=50.0)
                    # mask + apply on DVE fast paths (ts then 2-byte TT)
                    mk = workp.tile([128, 2 * W], f16, tag="mk")
                    nc.vector.tensor_scalar(
                        mk[:], utm, THR, None, op0=Alu.is_ge
                    )
                    ixm = workp.tile([128, 2 * W], f16, tag="ixm")
                    nc.vector.tensor_tensor(
                        out=ixm[:], in0=mk[:], in1=fm[:], op=Alu.mult
                    )

                    for jt in range(2):
                        base = kl * 256
                        # left: h=0 rows (i=p), stream reversed (i desc)
                        nc.tensor.matmul(
                            ps[jt][:, base:base + 128],
                            ixm[:, jt * 128:jt * 128 + 128],
                            idl[:],
                            start=False, stop=False, skip_group_check=True,
                        )
                        # right: h=1 rows (i=128+p), idr[0,0]=0 kills i=128
                        nc.tensor.matmul(
                            ps[jt][:, base + 128:base + 256],
                            ixm[:, 256 + jt * 128:256 + jt * 128 + 128],
                            idr[:],
                            start=False, stop=(kl == GK - 1),
                            skip_group_check=True,
                        )

                if pend is not None:
                    emit_group_tail(*pend)
                pend = (g, ps)
            emit_group_tail(*pend)

            for jt in range(2):
                # f32 loss: tbl -> f32 (ACT), t = tbl - refs, a = |t - 256|
                tblf = lossp.tile([128, REFW], f32, tag=f"tblf{jt}")
                nc.scalar.activation(tblf[:], tbl[jt][:], Act.Copy)
                t = lossp.tile([128, REFW], f32, tag=f"t{jt}")
                nc.vector.tensor_tensor(
                    out=t[:], in0=tblf[:], in1=refs_sb[jt][:],
                    op=Alu.subtract,
                )
                a = lossp.tile([128, REFW], f32, tag=f"a{jt}")
                nc.scalar.activation(a[:], t[:], Act.Abs, bias=noff[:, :])
                # ts+accum semantics: out=(in op0 s1); accum=reduce(out,op1,s2)
                # sum(a*[a<12]) = sum(min(a,12)) - 12*(TOT - N)
                jn = lossp.tile([128, REFW], f32, tag=f"jn{jt}")
                cnt = lossp.tile([128, 1], f32, tag=f"cnt{jt}")
                nc.vector.tensor_scalar(
                    jn[:], a[:], 12.0, 0.0, op0=Alu.is_lt, op1=Alu.add,
                    accum_out=cnt[:],
                )
                jz = lossp.tile([128, REFW], f32, tag=f"jz{jt}")
                acz = lossp.tile([128, 1], f32, tag=f"acz{jt}")
                nc.vector.tensor_scalar(
                    jz[:], a[:], 12.0, 0.0, op0=Alu.min, op1=Alu.add,
                    accum_out=acz[:],
                )
                nc.scalar.dma_start(out[:, 2 * jt:2 * jt + 1], cnt[:])
                nc.scalar.dma_start(out[:, 2 * jt + 1:2 * jt + 2], acz[:])

    nc.compile()
    return nc


def _host_constants():
    n = np.arange(128)
    veeL = (n + 1 + int(OFF)).astype(np.int16)
    veeR = (n + int(OFF)).astype(np.int16)
    vee256 = np.concatenate([veeL, veeR])
    vee = np.ascontiguousarray(
        np.broadcast_to(np.tile(vee256, GK), (128, PSW))
    ).astype(np.int16)

    ident = np.eye(128, dtype=np.float16)
    idl = np.ascontiguousarray(ident[::-1, :])  # anti-diagonal
    idr = ident.copy()
    idr[0, 0] = 0.0  # kill i=128

    ones1 = np.ones((1, 128), np.float16)
    npos = np.arange(PSW)
    patt = (TBLK * (npos // 256) + 51 * ((npos % 256) >= 128)).astype(
        np.float16
    )[None, :]
    return vee, idl, idr, ones1, np.ascontiguousarray(patt)


def _prep_refs(left, right):
    """[W, REFW] f32 per core: row (jt*128+j), col 102*k + c:
    c=0,51 trash(4000); c in [1,50] left bins 111..160 (x60);
    c in [52,101] right bins 111..160 (x60)."""
    lft = left[:, 0, :, 111:161]   # [B, W, 50]
    rgt = right[:, 0, :, 111:161]
    refs = np.full((NCORES, KPC, W, TBLK), 4000.0, np.float32)
    refs[..., 1:51] = (60.0 * lft).reshape(NCORES, KPC, W, 50)
    refs[..., 52:102] = (60.0 * rgt).reshape(NCORES, KPC, W, 50)
    refs = refs.transpose(0, 2, 1, 3).reshape(NCORES, W, REFW)
    return np.ascontiguousarray(refs)


def make_in_maps(up, left, right):
    up = np.asarray(up, np.float32)
    left = np.asarray(left, np.float32)
    right = np.asarray(right, np.float32)
    vee, idl, idr, ones1, patt = _host_constants()
    refs = _prep_refs(left, right)
    in_maps = []
    for c in range(NCORES):
        upc = np.ascontiguousarray(
            up[c * KPC:(c + 1) * KPC, 0].reshape(KPC * H, W)
        )
        in_maps.append({
            "up_in": upc,
            "refs_in": refs[c],
            "vee_in": vee,
            "idl_in": idl,
            "idr_in": idr,
            "ones_in": ones1,
            "patt_in": patt,
        })
    return in_maps


def get_nc():
    if "nc" not in _cache:
        _cache["nc"] = _build_bass()
    return _cache["nc"]


def reduce_results(results):
    # per partition/jt: N = count(a<12), S = sum(min(a,12));
    # sum(a*[a<12]) = S - 12*(REFW - N)
    total = 0.0
    for r in results:
        o = r["out"].astype(np.float64)
        total += o[:, 1].sum() + o[:, 3].sum()
        total += 12.0 * (o[:, 0].sum() + o[:, 2].sum())
    total -= 12.0 * REFW * 128 * 2 * NCORES
    return np.float32(total / (60.0 * B * W * W))


def kernel(up, left, right):
    nc = get_nc()
    in_maps = make_in_maps(up, left, right)
    res = run_bass_kernel_spmd(nc, in_maps, core_ids=list(range(NCORES)))
    return reduce_results(res.results)
